# revision 30
# baseline (speedup 1.0000x reference)
"""Trainium2 Bass kernel for the sliding-window CNN problem.

Computes, for x[B=32, WORDS=512, E=256], W[1024, 1280], b[1024]:
    z[b,t,h] = sum_{w<5, e<256} x[b, t+w, e] * W[h, w*256+e]   (T = 508 windows)
    out[b,h] = relu(max_t z[b,t,h] + b[h])

Strategy: data-parallel over batch (4 batches per core, 8 cores).
Per core the window conv is 10 accumulating matmuls (5 window shifts x 2
feature chunks of 128) per [128h x 508t] PSUM tile; the window shift is a
free SBUF column offset on the moving operand.  fp16 operands (same PE
rate as bf16, ~8x better accuracy), fp32 PSUM accumulation.  Loop is
kc-outer over all 8 hidden chunks (8 PSUM banks in flight) so the PE's
weight-consumption rate stays below the DMA delivery rate and compute
overlaps the weight loads.  Steady state runs at the fp16 roofline
(508 moving columns + ~7 cycles per matmul, LDWEIGHTS fully overlapped).

Head: the first matmul can only start once the first x/w DMAs land
(DMA queues have ~1.5us spin-up after the framework preamble), and the
PE clock (HAM) ramps 0.65->1.2->2.4 GHz over the first few us of
sustained activity.  A short junk-matmul bridge keeps the PE busy from
the earliest possible point to the moment data lands, so the clock ramp
overlaps the DMA wait instead of following it.

Tail: all post-processing (max-reduce + bias+relu) on DVE via
tensor_scalar (no ScalarE activation -> no 1.3us ACT_TABLE_LOAD).  The
last hidden chunk of the last batch is computed in two half-T PSUM
groups so the final reduce covers 254 columns instead of 508, and the
final combine is a single DVE op: max(mx_b + bias, relu(mx_a + bias)).
"""

import numpy as np

import concourse.bacc as bacc
import concourse.mybir as mybir
import concourse.tile as tile
from concourse.bass_utils import run_bass_kernel_spmd

B, WORDS, E = 32, 512, 256
WIN = 5
HIDDEN = 1024
T = WORDS - WIN + 1          # 508 sliding windows
NCORES = 8
BPC = B // NCORES            # 4 batches per core
F = WIN * E                  # 1280 contraction features
KC = F // 128                # 10 contraction chunks
HC = HIDDEN // 128           # 8 hidden chunks
EC = E // 128                # 2 feature chunks per window position
# ec-major accumulation order: all ec0 window shifts, then ec1.  Cuts the
# input/weight bytes needed in the first microseconds roughly in half.
KC_ORDER = [0, 2, 4, 6, 8, 1, 3, 5, 7, 9]

NJUNK_BIG = 6                # N=512 junk matmuls that drive the HAM clock ramp
NJUNK = 16                   # N=64 filler junk topping the bridge (~53ns each)

FP16 = mybir.dt.float16
FP32 = mybir.dt.float32

_CACHE = {}


def _build():
    nc = bacc.Bacc(None, target_bir_lowering=False)
    # xT[p, b, ec, t] = x[b, t, ec*128+p]
    xT = nc.dram_tensor("xT", [128, BPC, EC, WORDS], FP16, kind="ExternalInput")
    # wT[p, kc, h] = W[h, kc*128+p]
    wT = nc.dram_tensor("wT", [128, KC, HIDDEN], FP16, kind="ExternalInput")
    bias = nc.dram_tensor("bias", [128, HC], FP32, kind="ExternalInput")
    # out[b, p, hc] = result for batch b, hidden unit hc*128+p
    out = nc.dram_tensor("out", [BPC, 128, HC], FP32, kind="ExternalOutput")

    with tile.TileContext(nc) as tc:
        with (
            tc.tile_pool(name="xin", bufs=1) as xpool,
            tc.tile_pool(name="wgt", bufs=1) as wpool,
            tc.tile_pool(name="ps", bufs=1, space="PSUM") as pspool,
            tc.tile_pool(name="post", bufs=2) as postpool,
            tc.tile_pool(name="mxp", bufs=4) as mxpool,
            tc.tile_pool(name="cst", bufs=1) as cstpool,
        ):
            # DMA plan.  Constraints learned from traces: each DMA trigger is
            # ~650ns of serialized descgen on its engine; the 6th+ trigger on
            # an engine gates on an earlier DMA's completion (semaphore
            # recycling); per-partition lines below 1KB transfer slowly.  So:
            # keep the first five triggers per engine for the critical path,
            # interleave the weight chunks across both queues in consumption
            # order, and push bulk-x/bias behind them on the scalar queue.
            # The matmuls consume contraction chunks in ec-major order
            # (KC_ORDER): all five ec0 window shifts first, then ec1.  This
            # halves the x-bytes and interleaves away half the w-bytes
            # needed in the first ~8us, when the DMA queues still run at
            # ~half their warm rate.
            #   sync:   x0/ec0, w2, w4, w6, w8, (out DMAs later)
            #   scalar: w0[0:512], w0[512:1024], x0/ec1, w1, w3, w5, w7, w9,
            #           x1, x2, x3, bias
            xt = [xpool.tile([128, EC * WORDS], FP16, tag="x_0", name="x_0")]
            wt = [wpool.tile([128, HIDDEN], FP16, tag=f"w_{kc}", name=f"w_{kc}")
                  for kc in range(KC)]
            nc.sync.dma_start(xt[0][:, 0:WORDS], xT[:, 0, 0])
            nc.scalar.dma_start(wt[0][:, 0:512], wT[:, 0, 0:512])
            nc.scalar.dma_start(wt[0][:, 512:HIDDEN], wT[:, 0, 512:HIDDEN])
            nc.scalar.dma_start(xt[0][:, WORDS:2 * WORDS], xT[:, 0, 1])
            for kc in (2, 4, 6, 8):
                nc.sync.dma_start(wt[kc][:], wT[:, kc])
            for kc in (1, 3, 5, 7, 9):
                nc.scalar.dma_start(wt[kc][:], wT[:, kc])
            for b in range(1, BPC):
                t = xpool.tile([128, EC * WORDS], FP16, tag=f"x_{b}", name=f"x_{b}")
                nc.scalar.dma_start(t[:], xT[:, b])
                xt.append(t)
            bias_sb = cstpool.tile([128, HC], FP32, tag="bias")
            nc.scalar.dma_start(bias_sb[:], bias[:])

            # PE warmup bridge: the PE can be busy from ~0.5us after the
            # preamble barrier, but the first input DMA only lands ~3us
            # later.  Junk matmuls fill that window so the HAM clock ramp
            # runs concurrently with the DMA wait.  The ramp only counts
            # long matmuls (traces show full clock ~0.5-2us after the first
            # N~512 matmul, no matter how long N=64 junk ran), so the bulk
            # of the bridge is N=512 junk, topped off with short filler.
            junk = cstpool.tile([128, 512], FP16, tag="junk")
            nc.gpsimd.memset(junk[:, 0:8], 0.0)
            ps_junk = pspool.tile([128, 512], FP32, tag="ps7", name="ps_junk")
            for _ in range(NJUNK_BIG):
                nc.tensor.matmul(
                    ps_junk[:], junk[:, 0:128], junk[:], start=True, stop=True
                )
            for _ in range(NJUNK):
                nc.tensor.matmul(
                    ps_junk[:, 0:64], junk[:, 0:128], junk[:, 0:64],
                    start=True, stop=True,
                )

            def emit_group(b, hc, ps, lo, hi):
                """All KC accumulating matmuls for psum group (b, hc),
                moving columns [lo, hi) of the T range."""
                for j, kc in enumerate(KC_ORDER):
                    w, ec = divmod(kc, EC)
                    base = ec * WORDS + w
                    nc.tensor.matmul(
                        ps[:],
                        wt[kc][:, hc * 128:(hc + 1) * 128],
                        xt[b][:, base + lo: base + hi],
                        start=(j == 0),
                        stop=(j == KC - 1),
                    )

            def emit_post(b, hc, ps, res):
                # Unique mx tile per (b, hc): a shared/double-buffered tag
                # makes batch b's reduce WAR-wait on batch b-2's consumer,
                # which cascades the whole post pipeline behind the PE.
                mx = mxpool.tile([128, 1], FP32, tag=f"mx{hc}", name=f"mx_{b}_{hc}")
                nc.vector.reduce_max(mx[:], ps[:], axis=mybir.AxisListType.X)
                nc.gpsimd.tensor_scalar(
                    res[:, hc:hc + 1], mx[:], bias_sb[:, hc:hc + 1], 0.0,
                    mybir.AluOpType.add, mybir.AluOpType.max,
                )

            for b in range(BPC - 1):
                # kc-outer: all 8 banks accumulate in parallel; the PE's
                # weight consumption rate stays below DMA delivery, so
                # compute starts as soon as the first weight block lands.
                ps = [
                    pspool.tile([128, T], FP32, tag=f"ps{hc}", name=f"ps_{b}_{hc}")
                    for hc in range(HC)
                ]
                res = postpool.tile([128, HC], FP32, tag="res", name=f"res_{b}")
                for j, kc in enumerate(KC_ORDER):
                    w, ec = divmod(kc, EC)
                    base = ec * WORDS + w
                    rhs = xt[b][:, base: base + T]
                    for hc in range(HC):
                        nc.tensor.matmul(
                            ps[hc][:],
                            wt[kc][:, hc * 128:(hc + 1) * 128],
                            rhs,
                            start=(j == 0),
                            stop=(j == KC - 1),
                        )
                for hc in range(HC):
                    emit_post(b, hc, ps[hc], res)
                nc.sync.dma_start(out[b], res[:])

            # Last batch: hc-outer so groups finish staggered and the
            # reduce/act chain overlaps the remaining matmuls.  The final
            # hidden chunk is split into two half-T psum groups so only a
            # 254-column reduce plus one small DVE op trails the last
            # matmul.  Results ship in slices so just 512B of DMA remains
            # at the end.
            b = BPC - 1
            res = postpool.tile([128, HC], FP32, tag="res", name="res_last")
            for hc in range(HC - 1):
                psl = pspool.tile([128, T], FP32, tag=f"ps{hc}", name=f"ps_l_{hc}")
                emit_group(b, hc, psl, 0, T)
                emit_post(b, hc, psl, res)
                if hc == 3:
                    nc.sync.dma_start(out[b, :, 0:4], res[:, 0:4])
            hc = HC - 1
            TH = T // 2
            psa = pspool.tile([128, TH], FP32, tag="ps7", name="ps_l7a")
            emit_group(b, hc, psa, 0, TH)
            mxa = postpool.tile([128, 1], FP32, tag="mxa", name="mx_l7a")
            nc.vector.reduce_max(mxa[:], psa[:], axis=mybir.AxisListType.X)
            # ra = relu(mxa + bias); final = max(mxb + bias, ra)
            ra = postpool.tile([128, 1], FP32, tag="ra", name="ra_l7")
            nc.vector.tensor_scalar(
                ra[:], mxa[:], bias_sb[:, hc:hc + 1], 0.0,
                mybir.AluOpType.add, mybir.AluOpType.max,
            )
            psb = pspool.tile([128, T - TH], FP32, tag="ps6", name="ps_l7b")
            emit_group(b, hc, psb, TH, T)
            nc.sync.dma_start(out[b, :, 4:HC - 1], res[:, 4:HC - 1])
            mxb = postpool.tile([128, 1], FP32, tag="mxb", name="mx_l7b")
            nc.vector.reduce_max(mxb[:], psb[:], axis=mybir.AxisListType.X)
            nc.vector.tensor_scalar(
                res[:, hc:hc + 1], mxb[:], bias_sb[:, hc:hc + 1], ra[:],
                mybir.AluOpType.add, mybir.AluOpType.max,
            )
            nc.scalar.dma_start(out[b, :, hc:hc + 1], res[:, hc:hc + 1])
    nc.finalize()
    return nc


def _prep(input, W, b):
    x = np.asarray(input, dtype=np.float32)
    # x[b, t, e] -> xT[p, b, ec, t] = x[b, t, ec*128+p]
    y = x.transpose(2, 0, 1).reshape(EC, 128, B, WORDS)      # [ec, p, b, t]
    xT = np.ascontiguousarray(y.transpose(1, 2, 0, 3)).astype(np.float16)  # [p,b,ec,t]
    # W[h, f] -> wT[p, kc, h] = W[h, kc*128+p]
    wt = np.asarray(W, dtype=np.float32).T.reshape(KC, 128, HIDDEN)  # [kc, p, h]
    wT = np.ascontiguousarray(wt.transpose(1, 0, 2)).astype(np.float16)  # [p, kc, h]
    # b[h] -> bias[p, hc] = b[hc*128+p]
    bias = np.ascontiguousarray(np.asarray(b, np.float32).reshape(HC, 128).T)
    return xT, wT, bias


def run(inputs, trace=False, **kwargs):
    if "nc" not in _CACHE:
        _CACHE["nc"] = _build()
    nc = _CACHE["nc"]
    xT, wT, bias = _prep(inputs["input"], inputs["W"], inputs["b"])
    in_maps = [
        {"xT": xT[:, c * BPC:(c + 1) * BPC], "wT": wT, "bias": bias}
        for c in range(NCORES)
    ]
    in_maps = [{k: np.ascontiguousarray(v) for k, v in m.items()} for m in in_maps]
    res = run_bass_kernel_spmd(nc, in_maps, list(range(NCORES)), trace=trace, **kwargs)
    # out[b, p, hc] -> full[core*BPC + b, hc*128 + p]
    parts = []
    for c in range(NCORES):
        o = res.results[c]["out"]              # [BPC, 128, HC]
        parts.append(o.transpose(0, 2, 1).reshape(BPC, HIDDEN))
    full = np.concatenate(parts, axis=0).astype(np.float32)
    return full, res


def kernel(**inputs):
    out, _ = run(inputs, trace=False)
    return out
